# revision 1
# baseline (speedup 1.0000x reference)
"""Causal self-attention (B=4, T=2048, C=768, H=6, D=128) on 8 trn2 NeuronCores.

Sharding: 24 (batch, head) units -> 8 cores, each core owns 1 batch x 3 heads.
Unshard: out[b] = partial[core 2b] + partial[core 2b+1]  (tensor-parallel sum).

v3 design notes:
  - bf16 on every PE operand; fp32 only in PSUM accumulators and the norm
    statistics (measured end-to-end rel err ~6.5e-3 vs the 2e-2 gate).
  - STAGE INTERLEAVING: stage 1 (QKV+rope+norm, vector-engine-heavy, PE
    ~40%) is emitted round-robin with the attention chunks (PE-heavy,
    vector ~30%) whose K/V prefix is already available: tiles 0-3 up
    front, then chunk qc's blocks interleaved with tiles 4qc+4..4qc+7.
    Every engine sees a mix of both workloads, the PE never starves on
    the QKV psum recycle, and the Tensor engine's DVFS pstate stays hot.
  - causal diagonal blocks compute only the live column range; the mask
    is a host-supplied [128,512] lower-triangle bf16 tile, one DVE mult.
  - softmax denominators accumulate at partitions 0/32/64 of one PSUM
    tile; one reciprocal per chunk; per-head rows staged to partition 0
    before partition_broadcast (broadcast from base!=0 reads garbage).
  - PSUM budget (8 banks): qkv[128,3,512] x1 + psT x1 + scores x2 +
    yps x1 + dps x1.
"""

import numpy as np
import ml_dtypes

import concourse.bacc as bacc
import concourse.bass as bass
import concourse.mybir as mybir
from concourse import tile
from concourse.bass_utils import run_bass_kernel_spmd

F32 = mybir.dt.float32
BF16 = mybir.dt.bfloat16
AF = mybir.ActivationFunctionType
ALU = mybir.AluOpType

B, T, C, H, D = 4, 2048, 768, 6, 128
HALF = D // 2
NH = 3            # heads per core
CT = C // 128     # 6 contraction tiles for projections
NT = T // 128     # 16 token tiles
QC = 512          # query-chunk width for attention
NQC = T // QC     # 4 chunks
SCALE = 1.0 / float(np.sqrt(D))
EPS = 1e-6

_CACHE = {}


def _build_nc():
    nc = bacc.Bacc("TRN2")

    xT = nc.dram_tensor("xT", [C, T], BF16, kind="ExternalInput")
    wqT = nc.dram_tensor("wqT", [C, NH * D], BF16, kind="ExternalInput")
    wkT = nc.dram_tensor("wkT", [C, NH * D], BF16, kind="ExternalInput")
    wvT = nc.dram_tensor("wvT", [C, NH * D], BF16, kind="ExternalInput")
    wpT = nc.dram_tensor("wpT", [NH * D, C], BF16, kind="ExternalInput")
    cosb = nc.dram_tensor("cosb", [T, HALF], BF16, kind="ExternalInput")
    sinb = nc.dram_tensor("sinb", [T, HALF], BF16, kind="ExternalInput")
    maskC = nc.dram_tensor("maskC", [128, QC], BF16, kind="ExternalInput")
    ident = nc.dram_tensor("ident", [128, 128], BF16, kind="ExternalInput")
    ones_in = nc.dram_tensor("ones_in", [128, 1], BF16, kind="ExternalInput")
    out = nc.dram_tensor("out", [T, C], F32, kind="ExternalOutput")

    with tile.TileContext(nc) as tc:
        with (
            tc.tile_pool(name="persist", bufs=1) as persist,
            tc.tile_pool(name="qkvbuf", bufs=1) as qkvbuf,
            tc.tile_pool(name="wbuf", bufs=1) as wbuf,
            tc.tile_pool(name="xch", bufs=3) as xpool,
            tc.tile_pool(name="rope", bufs=3) as rpool,
            tc.tile_pool(name="nrmp", bufs=3) as npool,
            tc.tile_pool(name="stat", bufs=4) as spool,
            tc.tile_pool(name="att", bufs=5) as apool,
            tc.tile_pool(name="acc", bufs=2) as accpool,
            tc.tile_pool(name="ybuf", bufs=2) as ypool,
            tc.tile_pool(name="obuf", bufs=3) as opool,
            tc.tile_pool(name="psQKV", bufs=1, space="PSUM") as psQKV,
            tc.tile_pool(name="psT", bufs=1, space="PSUM") as psT,
            tc.tile_pool(name="psS", bufs=2, space="PSUM") as psS,
            tc.tile_pool(name="psY", bufs=1, space="PSUM") as psY,
            tc.tile_pool(name="psD", bufs=1, space="PSUM") as psD,
        ):
            QT = qkvbuf.tile([128, NH, T], BF16)       # [d, h, t]
            KT = qkvbuf.tile([128, NH, T], BF16)       # [d, h, t]
            V = qkvbuf.tile([128, NT, NH * D], BF16)   # [s%128, s//128, h*D+d]
            ones = persist.tile([128, 1], BF16)
            idn = persist.tile([128, 128], BF16)
            mask = persist.tile([128, QC], BF16)
            wp_sb = persist.tile([128, NH, C], BF16)   # [d, h, c]

            wq_sb = wbuf.tile([128, CT, NH * D], BF16)
            wk_sb = wbuf.tile([128, CT, NH * D], BF16)
            wv_sb = wbuf.tile([128, CT, NH * D], BF16)
            # startup ordering: first-matmul deps (wq ci 0-1, x tile 0)
            # land first, the rest in first-use order.
            wqT_r = wqT.rearrange("(ci p) o -> p ci o", p=128)
            nc.sync.dma_start(wq_sb[:, 0:2], wqT_r[:, 0:2])

            xT_r = xT.rearrange("(ci p) (tt t) -> p ci tt t", p=128, t=128)
            xch0 = xpool.tile([128, CT, 128], BF16, tag="xch")
            nc.sync.dma_start(xch0[:], xT_r[:, :, 0, :])
            nc.sync.dma_start(wq_sb[:, 2:CT], wqT_r[:, 2:CT])
            nc.sync.dma_start(wk_sb[:], wkT.rearrange("(ci p) o -> p ci o", p=128))
            nc.sync.dma_start(wv_sb[:], wvT.rearrange("(ci p) o -> p ci o", p=128))

            cos_sb = wbuf.tile([128, NT, HALF], BF16)
            sin_sb = wbuf.tile([128, NT, HALF], BF16)
            nc.sync.dma_start(cos_sb[:], cosb.rearrange("(tt p) f -> p tt f", p=128))
            nc.sync.dma_start(sin_sb[:], sinb.rearrange("(tt p) f -> p tt f", p=128))
            nc.sync.dma_start(idn[:], ident[:])
            nc.sync.dma_start(mask[:], maskC[:])
            nc.sync.dma_start(ones[:], ones_in[:])
            nc.sync.dma_start(wp_sb[:], wpT.rearrange("(h p) c -> p h c", p=128))

            out_r = out.rearrange("(tt p) c -> p tt c", p=128)

            # ------------- stage 1 emitters (one token tile each) -----------
            pend_tp = [None]

            def emit_transposes():
                if pend_tp[0] is None:
                    return
                nrm, tt = pend_tp[0]
                pend_tp[0] = None
                for mi in range(2):
                    dstT = QT if mi == 0 else KT
                    tps = psT.tile([128, NH * D], BF16, tag="tp")
                    for h in range(NH):
                        nc.tensor.transpose(
                            tps[:, h * D:(h + 1) * D], nrm[:, mi, h], idn[:])
                    dst = dstT[:, :, tt * 128:(tt + 1) * 128]
                    src = tps[:].rearrange("p (h t) -> p h t", h=NH)
                    nc.scalar.copy(dst, src)

            def stage1_emit(tt):
                if tt == 0:
                    xch = xch0
                else:
                    xch = xpool.tile([128, CT, 128], BF16, tag="xch")
                    nc.sync.dma_start(xch[:], xT_r[:, :, tt, :])

                # q in bank 0, k in bank 1, v in bank 2 of one psum tile
                qkv = psQKV.tile([128, 3, 512], F32, tag="qkv")
                for ci in range(CT):
                    st_, sp_ = (ci == 0), (ci == CT - 1)
                    lhs = xch[:, ci, :]
                    nc.tensor.matmul(qkv[:, 0, 0:NH * D], lhs, wq_sb[:, ci, :],
                                     start=st_, stop=sp_, skip_group_check=True)
                    nc.tensor.matmul(qkv[:, 1, 0:NH * D], lhs, wk_sb[:, ci, :],
                                     start=st_, stop=sp_, skip_group_check=True)
                    nc.tensor.matmul(qkv[:, 2, 0:NH * D], lhs, wv_sb[:, ci, :],
                                     start=st_, stop=sp_, skip_group_check=True)

                emit_transposes()

                # V: straight copy PSUM -> SBUF (bf16) in natural [t, o]
                nc.scalar.copy(V[:, tt, :], qkv[:, 2, 0:NH * D])

                # rope: r = t (.) [c,c]  +  [b,a] (.) [s,-s]
                # Pool can't read PSUM: ACT parks q,k in SBUF bf16 first.
                qk_sb = rpool.tile([128, 2, NH, D], BF16, tag="qksb")
                nc.scalar.copy(qk_sb[:].rearrange("p m h f -> p m (h f)"),
                               qkv[:, 0:2, 0:NH * D])
                qk6 = qk_sb[:].rearrange("p m h f -> p (m h) f")
                a = qk6[:, :, 0:HALF]
                b = qk6[:, :, HALF:D]
                cos_b3 = cos_sb[:, tt].unsqueeze(1).broadcast_to((128, 2 * NH, HALF))
                sin_b3 = sin_sb[:, tt].unsqueeze(1).broadcast_to((128, 2 * NH, HALF))
                u = rpool.tile([128, 2, NH, D], BF16, tag="u")
                w = rpool.tile([128, 2, NH, D], BF16, tag="w")
                r = rpool.tile([128, 2, NH, D], BF16, tag="r")
                u6 = u[:].rearrange("p m h f -> p (m h) f")
                w6 = w[:].rearrange("p m h f -> p (m h) f")
                nc.gpsimd.tensor_tensor(u6[:, :, 0:HALF], a, cos_b3, op=ALU.mult)
                nc.gpsimd.tensor_tensor(u6[:, :, HALF:D], b, cos_b3, op=ALU.mult)
                nc.gpsimd.tensor_tensor(w6[:, :, 0:HALF], b, sin_b3, op=ALU.mult)
                nc.vector.scalar_tensor_tensor(
                    w6[:, :, HALF:D], a, -1.0, sin_b3, op0=ALU.mult, op1=ALU.mult)
                nc.vector.tensor_add(r[:], u[:], w[:])

                # stats: mean / unbiased std per (token, head)
                r6 = r[:].rearrange("p m h f -> p (m h) f")
                sums = spool.tile([128, 2 * NH], F32, tag="sums")
                nc.vector.tensor_reduce(sums[:], r6, axis=mybir.AxisListType.X, op=ALU.add)
                junk = rpool.tile([128, 2, NH, D], BF16, tag="junk")
                nc.gpsimd.tensor_mul(junk[:], r[:], r[:])
                sumsq = spool.tile([128, 2 * NH], F32, tag="sumsq")
                nc.vector.tensor_reduce(
                    sumsq[:], junk[:].rearrange("p m h f -> p (m h) f"),
                    axis=mybir.AxisListType.X, op=ALU.add)
                negmean = spool.tile([128, 2 * NH], F32, tag="negmean")
                nc.vector.tensor_scalar_mul(negmean[:], sums[:], -1.0 / D)
                var = spool.tile([128, 2 * NH], F32, tag="var")
                nc.gpsimd.tensor_mul(var[:], negmean[:], sums[:])
                nc.gpsimd.tensor_add(var[:], sumsq[:], var[:])
                stdv = spool.tile([128, 2 * NH], F32, tag="stdv")
                nc.scalar.activation(stdv[:], var[:], AF.Sqrt, scale=1.0 / (D - 1))
                nc.vector.tensor_scalar_add(stdv[:], stdv[:], EPS)
                rstd = spool.tile([128, 2 * NH], F32, tag="rstd")
                nc.vector.reciprocal(rstd[:], stdv[:])

                # normalize: nrm = (r + negmean) * rstd
                nrm = npool.tile([128, 2, NH, D], BF16, tag="nrm")
                for mi in range(2):
                    for h in range(NH):
                        c = mi * NH + h
                        nc.vector.tensor_scalar(
                            nrm[:, mi, h], r[:, mi, h],
                            negmean[:, c:c + 1], rstd[:, c:c + 1],
                            op0=ALU.add, op1=ALU.mult)

                pend_tp[0] = (nrm, tt)

            # ------------- attention chunk emitters (as quanta) -------------
            def emit_proj(qc, yTc):
                quanta = []
                for j in range(QC // 128):
                    def thunk(j=j, qc=qc, yTc=yTc):
                        tt = qc * (QC // 128) + j
                        op0 = psS.tile([128, C // 2], F32, tag="ps")
                        op1 = psS.tile([128, C // 2], F32, tag="ps")
                        for h in range(NH):
                            lhs = yTc[:, h, j * 128:(j + 1) * 128]
                            nc.tensor.matmul(op0[:], lhs, wp_sb[:, h, 0:C // 2],
                                             start=(h == 0), stop=(h == NH - 1))
                            nc.tensor.matmul(op1[:], lhs, wp_sb[:, h, C // 2:C],
                                             start=(h == 0), stop=(h == NH - 1))
                        ot = opool.tile([128, C], F32, tag="ot")
                        nc.scalar.copy(ot[:, 0:C // 2], op0[:])
                        nc.scalar.copy(ot[:, C // 2:C], op1[:])
                        nc.sync.dma_start(out_r[:, tt, :], ot[:])
                    quanta.append(thunk)
                return quanta

            def attn_quanta(qc, pending_proj):
                """List of emission thunks for chunk qc (+ prev chunk's proj)."""
                Q0 = qc * QC
                n_st = (Q0 + QC) // 128
                state = {}
                ets = {}

                def start_chunk():
                    state["dps"] = psD.tile([128, QC], F32, tag="dps", name="dps")
                    state["yU"] = ypool.tile([128, NH, QC], BF16, tag="yU", name="yU")

                def loc0_of(st):
                    j = st - (n_st - 4)
                    return 128 * j if j > 0 else 0

                def emit_score(h, st):
                    loc0 = loc0_of(st)
                    sps = psS.tile([128, QC], F32, tag="ps")
                    nc.tensor.matmul(
                        sps[:, loc0:QC],
                        KT[:, h, st * 128:(st + 1) * 128],
                        QT[:, h, Q0 + loc0:Q0 + QC],
                        start=True, stop=True)
                    et = apool.tile([128, QC], BF16, tag="et")
                    nc.scalar.activation(et[:, loc0:QC], sps[:, loc0:QC],
                                         AF.Exp, scale=SCALE)
                    if st * 128 >= Q0:  # diagonal block: zero where s > q
                        nc.vector.tensor_mul(et[:, loc0:QC], et[:, loc0:QC],
                                             mask[:, 0:QC - loc0])
                    ets[(h, st)] = et

                def emit_av(h, st):
                    loc0 = loc0_of(st)
                    et = ets.pop((h, st))
                    nc.tensor.matmul(
                        state["yps"][:, loc0:QC],
                        V[:, st, h * D:(h + 1) * D],
                        et[:, loc0:QC],
                        start=(st == 0), stop=(st == n_st - 1),
                        skip_group_check=True)
                    nc.tensor.matmul(
                        state["dps"][32 * h:32 * h + 1, loc0:QC],
                        ones[:],
                        et[:, loc0:QC],
                        start=(st == 0), stop=(st == n_st - 1),
                        skip_group_check=True)

                quanta = [start_chunk]
                for h in range(NH):
                    def head_start(h=h):
                        state["yps"] = psY.tile([128, QC], F32, tag="yps", name="yps")
                        emit_score(h, 0)
                        if n_st > 1:
                            emit_score(h, 1)
                    quanta.append(head_start)
                    for st in range(n_st):
                        def block(h=h, st=st):
                            if st + 2 < n_st:
                                emit_score(h, st + 2)
                            emit_av(h, st)
                        quanta.append(block)
                    if h == 0 and pending_proj:
                        quanta.extend(pending_proj)

                    def head_end(h=h):
                        # park unnormalized y in SBUF so the psum bank frees
                        nc.vector.tensor_copy(state["yU"][:, h, :], state["yps"][:])
                    quanta.append(head_end)

                def chunk_end():
                    dps, yU = state["dps"], state["yU"]
                    rc = accpool.tile([128, QC], BF16, tag="rc")
                    with nc.allow_low_precision(reason="bf16 softmax denom"):
                        nc.vector.reciprocal(rc[0:65, :], dps[0:65, :])
                    yTc = ypool.tile([128, NH, QC], BF16, tag="yT")
                    for h in range(NH):
                        # broadcast reads garbage from base partition != 0:
                        # stage each head's row at partition 0 first
                        rch = accpool.tile([1, QC], BF16, tag="rch")
                        nc.vector.tensor_copy(rch[:], rc[32 * h:32 * h + 1, :])
                        rbc = accpool.tile([128, QC], BF16, tag="rbc")
                        nc.gpsimd.partition_broadcast(rbc[:], rch[:])
                        nc.vector.tensor_mul(yTc[:, h, :], yU[:, h, :], rbc[:])
                    state["yTc"] = yTc
                quanta.append(chunk_end)
                return quanta, state

            # ------------- interleaved emission schedule --------------------
            for tt in range(4):
                stage1_emit(tt)

            pending_proj = None
            for qc in range(NQC):
                emit_transposes()  # chunk qc needs tiles <= 4qc+3 fully out
                quanta, state = attn_quanta(qc, pending_proj)
                tiles = list(range(4 * qc + 4, min(4 * qc + 8, NT)))
                # spread the next chunk's stage-1 tiles among this chunk's
                # attention quanta so every engine sees both workloads
                stride = max(1, len(quanta) // (len(tiles) + 1)) if tiles else 0
                ti = 0
                for i, q in enumerate(quanta):
                    q()
                    if tiles and ti < len(tiles) and i % stride == stride - 1:
                        stage1_emit(tiles[ti])
                        ti += 1
                while ti < len(tiles):
                    stage1_emit(tiles[ti])
                    ti += 1
                pending_proj = emit_proj(qc, state["yTc"])
            for q in pending_proj:
                q()

    nc.compile()
    return nc


def _get_nc():
    if "nc" not in _CACHE:
        _CACHE["nc"] = _build_nc()
    return _CACHE["nc"]


def _in_maps(x, cos, sin, wq, wk, wv, wproj):
    bf = ml_dtypes.bfloat16
    cosb = np.ascontiguousarray(np.asarray(cos).astype(bf))
    sinb = np.ascontiguousarray(np.asarray(sin).astype(bf))
    maskC = np.ascontiguousarray(
        (np.arange(QC)[None, :] >= np.arange(128)[:, None]).astype(bf))
    ident = np.eye(128, dtype=bf)
    maps = []
    for c in range(8):
        b = c // 2
        hs = (c % 2) * NH
        sl = slice(hs * D, (hs + NH) * D)
        maps.append({
            "xT": np.ascontiguousarray(np.asarray(x[b]).T.astype(bf)),
            "wqT": np.ascontiguousarray(np.asarray(wq)[sl].T.astype(bf)),
            "wkT": np.ascontiguousarray(np.asarray(wk)[sl].T.astype(bf)),
            "wvT": np.ascontiguousarray(np.asarray(wv)[sl].T.astype(bf)),
            "wpT": np.ascontiguousarray(np.asarray(wproj).T[sl].astype(bf)),
            "cosb": cosb,
            "sinb": sinb,
            "maskC": maskC,
            "ident": ident,
            "ones_in": np.ones((128, 1), dtype=bf),
        })
    return maps


def kernel(x, cos, sin, wq, wk, wv, wproj, _trace=False):
    nc = _get_nc()
    maps = _in_maps(x, cos, sin, wq, wk, wv, wproj)
    res = run_bass_kernel_spmd(nc, maps, core_ids=list(range(8)), trace=_trace)
    parts = [r["out"] for r in res.results]
    outv = np.stack([parts[2 * b] + parts[2 * b + 1] for b in range(B)]).astype(np.float32)
    if _trace:
        _CACHE["last_results"] = res
    return outv



# revision 2
# speedup vs baseline: 1.1640x; 1.1640x over previous
"""Causal self-attention (B=4, T=2048, C=768, H=6, D=128) on 8 trn2 NeuronCores.

Sharding: 24 (batch, head) units -> 8 cores, each core owns 1 batch x 3 heads.
Unshard: out[b] = partial[core 2b] + partial[core 2b+1]  (tensor-parallel sum).

v4 design notes (vs v3):
  - TWO-PHASE schedule. v3 interleaved stage 1 (QKV+rope+norm) with the
    attention chunks just-in-time; the trace showed 114us of PE idle (one
    30us stall waiting for the rope/norm chain) and HAM clock throttling
    from the gaps. Phase 1 now runs all 16 token tiles of QKV+rope+norm+
    transpose with DOUBLE-BUFFERED qkv psum (6 banks) + psT (2 banks);
    phase 2 is pure attention+proj with psS 3 / psY 2 / psD 1 / psP 2.
  - Phase separation also kills the ACT_TABLE_LOAD thrash (32us in v3):
    phase 1 runs Sqrt+Copy only, phase 2 Exp only.
  - rope via host-precomputed cosr=[c|c]x3, sinr=[s|-s]x3 ([T,384] each):
    r = qk (.) cosr + swap(qk) (.) sinr, all contiguous 2x-mode DVE ops;
    the half-swap is materialized by gpsimd from qk_sb. No broadcast_to
    operands (v3's gpsimd rope muls ran at 54 G/s against a 153 peak).
  - engine budget per phase-1 tile: PE 3.2us, ACT ~2.2, DVE ~2.5,
    gpsimd ~2.6 -> PE-bound. Phase 2: PE ~80us, ACT exp ~71 -> PE-bound.
  - softmax denominator reciprocal via reciprocal_approx_fast (fp32,
    ~18 bits; exact reciprocal's iterative divide cost 4us per chunk).
  - output in bf16 (halves the out DMA); host sums the TP pairs in fp32.
"""

import numpy as np
import ml_dtypes

import concourse.bacc as bacc
import concourse.bass as bass
import concourse.mybir as mybir
from concourse import tile
from concourse.bass_utils import run_bass_kernel_spmd

F32 = mybir.dt.float32
BF16 = mybir.dt.bfloat16
AF = mybir.ActivationFunctionType
ALU = mybir.AluOpType

B, T, C, H, D = 4, 2048, 768, 6, 128
HALF = D // 2
NH = 3            # heads per core
CT = C // 128     # 6 contraction tiles for projections
NT = T // 128     # 16 token tiles
QC = 512          # query-chunk width for attention
NQC = T // QC     # 4 chunks
SCALE = 1.0 / float(np.sqrt(D))
EPS = 1e-6
LAG = 4           # token tiles between QKV emission and its transposes

_CACHE = {}


def _build_nc():
    nc = bacc.Bacc("TRN2")

    xT = nc.dram_tensor("xT", [C, T], BF16, kind="ExternalInput")
    wqT = nc.dram_tensor("wqT", [C, NH * D], BF16, kind="ExternalInput")
    wkT = nc.dram_tensor("wkT", [C, NH * D], BF16, kind="ExternalInput")
    wvT = nc.dram_tensor("wvT", [C, NH * D], BF16, kind="ExternalInput")
    wpT = nc.dram_tensor("wpT", [NH * D, C], BF16, kind="ExternalInput")
    cosr = nc.dram_tensor("cosr", [T, NH * D], BF16, kind="ExternalInput")
    sinr = nc.dram_tensor("sinr", [T, NH * D], BF16, kind="ExternalInput")
    maskC = nc.dram_tensor("maskC", [128, QC], BF16, kind="ExternalInput")
    ident = nc.dram_tensor("ident", [128, 128], BF16, kind="ExternalInput")
    ones_in = nc.dram_tensor("ones_in", [128, 1], BF16, kind="ExternalInput")
    out = nc.dram_tensor("out", [T, C], BF16, kind="ExternalOutput")

    with tile.TileContext(nc) as tc:
        with (
            tc.tile_pool(name="persist", bufs=1) as persist,
            tc.tile_pool(name="qkvbuf", bufs=1) as qkvbuf,
            tc.tile_pool(name="wbuf", bufs=1) as wbuf,
        ):
            QT = qkvbuf.tile([128, NH, T], BF16)       # [d, h, t]
            KT = qkvbuf.tile([128, NH, T], BF16)       # [d, h, t]
            V = qkvbuf.tile([128, NT, NH * D], BF16)   # [s%128, s//128, h*D+d]
            ones = persist.tile([128, 1], BF16)
            idn = persist.tile([128, 128], BF16)
            mask = persist.tile([128, QC], BF16)
            wp_sb = persist.tile([128, NH, C], BF16)   # [d, h, c]

            wq_sb = wbuf.tile([128, CT, NH * D], BF16)
            wk_sb = wbuf.tile([128, CT, NH * D], BF16)
            wv_sb = wbuf.tile([128, CT, NH * D], BF16)
            x_sb = wbuf.tile([128, CT, T], BF16)       # [c%128, c//128, t]

            # startup ordering: first-matmul deps (wq, x tiles 0-3) first.
            wqT_r = wqT.rearrange("(ci p) o -> p ci o", p=128)
            nc.sync.dma_start(wq_sb[:], wqT_r[:])
            xT_r = xT.rearrange("(ci p) (g t) -> p ci g t", p=128, g=4)
            for g in range(4):
                nc.sync.dma_start(
                    x_sb[:].rearrange("p ci (g t) -> p ci g t", g=4)[:, :, g],
                    xT_r[:, :, g])
            nc.sync.dma_start(wk_sb[:], wkT.rearrange("(ci p) o -> p ci o", p=128))
            nc.sync.dma_start(wv_sb[:], wvT.rearrange("(ci p) o -> p ci o", p=128))

            cos_sb = wbuf.tile([128, NT, NH * D], BF16)
            sin_sb = wbuf.tile([128, NT, NH * D], BF16)
            nc.sync.dma_start(cos_sb[:], cosr.rearrange("(tt p) f -> p tt f", p=128))
            nc.sync.dma_start(sin_sb[:], sinr.rearrange("(tt p) f -> p tt f", p=128))
            nc.sync.dma_start(idn[:], ident[:])
            nc.sync.dma_start(mask[:], maskC[:])
            nc.sync.dma_start(ones[:], ones_in[:])
            nc.sync.dma_start(wp_sb[:], wpT.rearrange("(h p) c -> p h c", p=128))

            out_r = out.rearrange("(tt p) c -> p tt c", p=128)

            # ---------------- phase 1: QKV + rope + norm + transpose --------
            nrm_of = {}
            with (
                tc.tile_pool(name="qkp", bufs=3) as qkpool,
                tc.tile_pool(name="rope", bufs=3) as rpool,
                tc.tile_pool(name="nrmp", bufs=LAG + 2) as npool,
                tc.tile_pool(name="stat", bufs=4) as spool,
                tc.tile_pool(name="psQKV", bufs=2, space="PSUM") as psQKV,
                tc.tile_pool(name="psT", bufs=2, space="PSUM") as psT,
            ):
                def stage1_emit(tt):
                    # q in bank 0, k in bank 1, v in bank 2 of one psum tile
                    qkv = psQKV.tile([128, 3, 512], F32, tag="qkv")
                    for ci in range(CT):
                        st_, sp_ = (ci == 0), (ci == CT - 1)
                        lhs = x_sb[:, ci, tt * 128:(tt + 1) * 128]
                        nc.tensor.matmul(qkv[:, 0, 0:NH * D], lhs, wq_sb[:, ci, :],
                                         start=st_, stop=sp_, skip_group_check=True)
                        nc.tensor.matmul(qkv[:, 1, 0:NH * D], lhs, wk_sb[:, ci, :],
                                         start=st_, stop=sp_, skip_group_check=True)
                        nc.tensor.matmul(qkv[:, 2, 0:NH * D], lhs, wv_sb[:, ci, :],
                                         start=st_, stop=sp_, skip_group_check=True)

                    # V: straight copy PSUM -> SBUF (bf16) in natural [t, o]
                    nc.scalar.copy(V[:, tt, :], qkv[:, 2, 0:NH * D])

                    # park q,k in SBUF bf16 (ACT reads PSUM)
                    qk_sb = qkpool.tile([128, 2, NH, D], BF16, tag="qksb")
                    nc.scalar.copy(qk_sb[:].rearrange("p m h f -> p m (h f)"),
                                   qkv[:, 0:2, 0:NH * D])

                    # half-swapped copy for rope (gpsimd, SBUF->SBUF)
                    qksw = qkpool.tile([128, 2, NH, D], BF16, tag="qksw")
                    nc.gpsimd.tensor_copy(qksw[:, :, :, 0:HALF],
                                          qk_sb[:, :, :, HALF:D])
                    nc.gpsimd.tensor_copy(qksw[:, :, :, HALF:D],
                                          qk_sb[:, :, :, 0:HALF])

                    # rope: r = qk (.) [c|c] + swap(qk) (.) [s|-s]
                    u = rpool.tile([128, 2, NH, D], BF16, tag="u")
                    w = rpool.tile([128, 2, NH, D], BF16, tag="w")
                    r = rpool.tile([128, 2, NH, D], BF16, tag="r")
                    for m in range(2):
                        nc.vector.tensor_mul(
                            u[:, m].rearrange("p h f -> p (h f)"),
                            qk_sb[:, m].rearrange("p h f -> p (h f)"),
                            cos_sb[:, tt])
                        nc.vector.tensor_mul(
                            w[:, m].rearrange("p h f -> p (h f)"),
                            qksw[:, m].rearrange("p h f -> p (h f)"),
                            sin_sb[:, tt])
                    nc.vector.tensor_add(r[:], u[:], w[:])

                    # stats: mean / unbiased std per (token, head)
                    r6 = r[:].rearrange("p m h f -> p (m h) f")
                    sums = spool.tile([128, 2 * NH], F32, tag="sums")
                    nc.vector.tensor_reduce(sums[:], r6, axis=mybir.AxisListType.X,
                                            op=ALU.add)
                    junk = rpool.tile([128, 2, NH, D], BF16, tag="junk")
                    nc.gpsimd.tensor_mul(junk[:], r[:], r[:])
                    sumsq = spool.tile([128, 2 * NH], F32, tag="sumsq")
                    nc.vector.tensor_reduce(
                        sumsq[:], junk[:].rearrange("p m h f -> p (m h) f"),
                        axis=mybir.AxisListType.X, op=ALU.add)
                    negmean = spool.tile([128, 2 * NH], F32, tag="negmean")
                    nc.gpsimd.tensor_scalar_mul(negmean[:], sums[:], -1.0 / D)
                    var = spool.tile([128, 2 * NH], F32, tag="var")
                    nc.gpsimd.tensor_mul(var[:], negmean[:], sums[:])
                    nc.gpsimd.tensor_add(var[:], sumsq[:], var[:])
                    stdv = spool.tile([128, 2 * NH], F32, tag="stdv")
                    nc.scalar.activation(stdv[:], var[:], AF.Sqrt, scale=1.0 / (D - 1))
                    nc.gpsimd.tensor_scalar_add(stdv[:], stdv[:], EPS)
                    rstd = spool.tile([128, 2 * NH], F32, tag="rstd")
                    nc.vector.reciprocal(rstd[:], stdv[:])

                    # normalize: nrm = (r + negmean) * rstd  (q on DVE, k on gpsimd)
                    nrm = npool.tile([128, 2, NH, D], BF16, tag="nrm")
                    for mi in range(2):
                        eng = nc.vector if mi == 0 else nc.gpsimd
                        for h in range(NH):
                            c = mi * NH + h
                            eng.tensor_scalar(
                                nrm[:, mi, h], r[:, mi, h],
                                negmean[:, c:c + 1], rstd[:, c:c + 1],
                                op0=ALU.add, op1=ALU.mult)
                    nrm_of[tt] = nrm

                def emit_transposes(tt):
                    nrm = nrm_of.pop(tt)
                    for mi in range(2):
                        dstT = QT if mi == 0 else KT
                        tps = psT.tile([128, NH * D], BF16, tag="tp")
                        for h in range(NH):
                            nc.tensor.transpose(
                                tps[:, h * D:(h + 1) * D], nrm[:, mi, h], idn[:])
                        dst = dstT[:, :, tt * 128:(tt + 1) * 128]
                        src = tps[:].rearrange("p (h t) -> p h t", h=NH)
                        nc.scalar.copy(dst, src)

                for tt in range(NT):
                    stage1_emit(tt)
                    if tt >= LAG:
                        emit_transposes(tt - LAG)
                for tt in range(NT - LAG, NT):
                    emit_transposes(tt)

            # ---------------- phase 2: attention + proj ---------------------
            with (
                tc.tile_pool(name="att", bufs=5) as apool,
                tc.tile_pool(name="acc", bufs=2) as accpool,
                tc.tile_pool(name="ybuf", bufs=2) as ypool,
                tc.tile_pool(name="obuf", bufs=3) as opool,
                tc.tile_pool(name="psS", bufs=3, space="PSUM") as psS,
                tc.tile_pool(name="psY", bufs=2, space="PSUM") as psY,
                tc.tile_pool(name="psD", bufs=1, space="PSUM") as psD,
                tc.tile_pool(name="psP", bufs=2, space="PSUM") as psP,
            ):
                def emit_proj(qc, yTc):
                    quanta = []
                    for j in range(QC // 128):
                        def thunk(j=j, qc=qc, yTc=yTc):
                            tt = qc * (QC // 128) + j
                            op0 = psP.tile([128, C // 2], F32, tag="pp")
                            op1 = psP.tile([128, C // 2], F32, tag="pp")
                            for h in range(NH):
                                lhs = yTc[:, h, j * 128:(j + 1) * 128]
                                nc.tensor.matmul(op0[:], lhs, wp_sb[:, h, 0:C // 2],
                                                 start=(h == 0), stop=(h == NH - 1))
                                nc.tensor.matmul(op1[:], lhs, wp_sb[:, h, C // 2:C],
                                                 start=(h == 0), stop=(h == NH - 1))
                            ot = opool.tile([128, C], BF16, tag="ot")
                            nc.vector.tensor_copy(ot[:, 0:C // 2], op0[:])
                            nc.vector.tensor_copy(ot[:, C // 2:C], op1[:])
                            nc.sync.dma_start(out_r[:, tt, :], ot[:])
                        quanta.append(thunk)
                    return quanta

                def attn_quanta(qc, pending_proj):
                    """List of emission thunks for chunk qc (+ prev chunk's proj)."""
                    Q0 = qc * QC
                    n_st = (Q0 + QC) // 128
                    state = {}
                    ets = {}

                    def start_chunk():
                        state["dps"] = psD.tile([128, QC], F32, tag="dps", name="dps")
                        state["yU"] = ypool.tile([128, NH, QC], BF16, tag="yU",
                                                 name="yU")

                    def loc0_of(st):
                        j = st - (n_st - 4)
                        return 128 * j if j > 0 else 0

                    def emit_score(h, st):
                        loc0 = loc0_of(st)
                        sps = psS.tile([128, QC], F32, tag="ps")
                        nc.tensor.matmul(
                            sps[:, loc0:QC],
                            KT[:, h, st * 128:(st + 1) * 128],
                            QT[:, h, Q0 + loc0:Q0 + QC],
                            start=True, stop=True)
                        et = apool.tile([128, QC], BF16, tag="et")
                        nc.scalar.activation(et[:, loc0:QC], sps[:, loc0:QC],
                                             AF.Exp, scale=SCALE)
                        if st * 128 >= Q0:  # diagonal block: zero where s > q
                            nc.vector.tensor_mul(et[:, loc0:QC], et[:, loc0:QC],
                                                 mask[:, 0:QC - loc0])
                        ets[(h, st)] = et

                    def emit_av(h, st):
                        loc0 = loc0_of(st)
                        et = ets.pop((h, st))
                        nc.tensor.matmul(
                            state["yps"][:, loc0:QC],
                            V[:, st, h * D:(h + 1) * D],
                            et[:, loc0:QC],
                            start=(st == 0), stop=(st == n_st - 1),
                            skip_group_check=True)
                        nc.tensor.matmul(
                            state["dps"][32 * h:32 * h + 1, loc0:QC],
                            ones[:],
                            et[:, loc0:QC],
                            start=(st == 0), stop=(st == n_st - 1),
                            skip_group_check=True)

                    quanta = [start_chunk]
                    for h in range(NH):
                        def head_start(h=h):
                            state["yps"] = psY.tile([128, QC], F32, tag="yps",
                                                    name="yps")
                            emit_score(h, 0)
                            if n_st > 1:
                                emit_score(h, 1)
                        quanta.append(head_start)
                        for st in range(n_st):
                            def block(h=h, st=st):
                                if st + 2 < n_st:
                                    emit_score(h, st + 2)
                                emit_av(h, st)
                            quanta.append(block)
                        if h == 0 and pending_proj:
                            quanta.extend(pending_proj)

                        def head_end(h=h):
                            # park unnormalized y in SBUF so the psum bank frees
                            nc.vector.tensor_copy(state["yU"][:, h, :],
                                                  state["yps"][:])
                        quanta.append(head_end)

                    def chunk_end():
                        dps, yU = state["dps"], state["yU"]
                        rcf = accpool.tile([65, QC], F32, tag="rcf")
                        nc.vector.reciprocal_approx_fast(rcf[:], dps[0:65, :])
                        yTc = ypool.tile([128, NH, QC], BF16, tag="yT")
                        for h in range(NH):
                            # broadcast reads garbage from base partition != 0:
                            # stage each head's row at partition 0 first
                            rch = accpool.tile([1, QC], BF16, tag="rch")
                            nc.vector.tensor_copy(rch[:], rcf[32 * h:32 * h + 1, :])
                            rbc = accpool.tile([128, QC], BF16, tag="rbc")
                            nc.gpsimd.partition_broadcast(rbc[:], rch[:])
                            nc.vector.tensor_mul(yTc[:, h, :], yU[:, h, :], rbc[:])
                        state["yTc"] = yTc
                    quanta.append(chunk_end)
                    return quanta, state

                pending_proj = None
                for qc in range(NQC):
                    quanta, state = attn_quanta(qc, pending_proj)
                    for q in quanta:
                        q()
                    pending_proj = emit_proj(qc, state["yTc"])
                for q in pending_proj:
                    q()

    nc.compile()
    return nc


def _get_nc():
    if "nc" not in _CACHE:
        _CACHE["nc"] = _build_nc()
    return _CACHE["nc"]


def _in_maps(x, cos, sin, wq, wk, wv, wproj):
    bf = ml_dtypes.bfloat16
    cos_np = np.asarray(cos, dtype=np.float32)
    sin_np = np.asarray(sin, dtype=np.float32)
    cosr = np.ascontiguousarray(
        np.tile(np.concatenate([cos_np, cos_np], axis=1), (1, NH)).astype(bf))
    sinr = np.ascontiguousarray(
        np.tile(np.concatenate([sin_np, -sin_np], axis=1), (1, NH)).astype(bf))
    maskC = np.ascontiguousarray(
        (np.arange(QC)[None, :] >= np.arange(128)[:, None]).astype(bf))
    ident = np.eye(128, dtype=bf)
    maps = []
    for c in range(8):
        b = c // 2
        hs = (c % 2) * NH
        sl = slice(hs * D, (hs + NH) * D)
        maps.append({
            "xT": np.ascontiguousarray(np.asarray(x[b]).T.astype(bf)),
            "wqT": np.ascontiguousarray(np.asarray(wq)[sl].T.astype(bf)),
            "wkT": np.ascontiguousarray(np.asarray(wk)[sl].T.astype(bf)),
            "wvT": np.ascontiguousarray(np.asarray(wv)[sl].T.astype(bf)),
            "wpT": np.ascontiguousarray(np.asarray(wproj).T[sl].astype(bf)),
            "cosr": cosr,
            "sinr": sinr,
            "maskC": maskC,
            "ident": ident,
            "ones_in": np.ones((128, 1), dtype=bf),
        })
    return maps


def kernel(x, cos, sin, wq, wk, wv, wproj, _trace=False):
    nc = _get_nc()
    maps = _in_maps(x, cos, sin, wq, wk, wv, wproj)
    res = run_bass_kernel_spmd(nc, maps, core_ids=list(range(8)), trace=_trace)
    parts = [np.asarray(r["out"], dtype=np.float32) for r in res.results]
    outv = np.stack([parts[2 * b] + parts[2 * b + 1] for b in range(B)])
    if _trace:
        _CACHE["last_results"] = res
    return outv
